# revision 1
# baseline (speedup 1.0000x reference)
"""DetectionLoss Trainium2 kernel: 8-core data-parallel (4 images/core).

Device computes, per image, partial sums over anchors ([128,6] per image):
  [npos, nneg, sum(ce_bg), sum(ce_tgt*posf), sum(ce_bg*negf), -sum(sl*posf)]
Host finishes the scalar combine exactly as the reference does.

Matched-GT gather runs on the PE: transpose the argmax tie-mask, then
matmul against per-GT [gcx, gcy, log(gw), log(gh), label] columns.
"""
import os
import sys
import numpy as np

sys.path.insert(0, "/opt/trn_rl_repo")

import concourse.bass as bass
import concourse.bacc as bacc
import concourse.mybir as mybir
from concourse import tile
from concourse.bass_utils import run_bass_kernel_spmd

F32 = mybir.dt.float32
ALU = mybir.AluOpType
ACT = mybir.ActivationFunctionType
AX = mybir.AxisListType

P = 128          # partitions
G = 200          # free columns per anchor plane (N = P*G = 25600)
N = P * G
M = 50           # max GT boxes
C = 8            # classes
BPC = 4          # images per core
NCORES = 8
GPC = 25         # groups per pair-stage chunk
NCHUNK = G // GPC
NQ = 5           # gathered per-GT quantities

# anchor plane indices in the "anc" DRAM tensor
A_CXM, A_CYM, A_WH, A_HH, A_W, A_H, A_I2W, A_I2H, A_LW, A_LH, A_CX, A_CY = range(12)
NANC = 12
# gt plane indices (each M wide) in "gt"
G_X1, G_Y1, G_X2, G_Y2, G_A2 = range(5)
NGT = 5
NOUT = 6


def _rep_last(ap, n):
    """[..., d] -> [..., d, n] with step-0 broadcast."""
    return bass.AP(ap.tensor, ap.offset, list(ap.ap) + [[0, n]])


def _rep_mid(ap, n):
    """[p, d] -> [p, n, d] with step-0 broadcast."""
    a = list(ap.ap)
    return bass.AP(ap.tensor, ap.offset, [a[0], [0, n]] + a[1:])


def _v3(ap2d):
    """[128, 200] plane -> [128, 8, 25]."""
    return ap2d.rearrange("p (u v) -> p u v", v=GPC)


def build_program():
    nc = bacc.Bacc(None, target_bir_lowering=False)
    cls_d = nc.dram_tensor("cls", [BPC, P, C * G], F32, kind="ExternalInput")
    reg_d = nc.dram_tensor("reg", [BPC, P, 4 * G], F32, kind="ExternalInput")
    anc_d = nc.dram_tensor("anc", [P, NANC * G], F32, kind="ExternalInput")
    gt_d = nc.dram_tensor("gt", [BPC, P, NGT * M], F32, kind="ExternalInput")
    gt5_d = nc.dram_tensor("gt5", [BPC, 64, 8], F32, kind="ExternalInput")
    iden_d = nc.dram_tensor("iden", [P, P], F32, kind="ExternalInput")
    res_d = nc.dram_tensor("res", [BPC, P, NOUT], F32, kind="ExternalOutput")

    with tile.TileContext(nc) as tc:
        with (
            tc.tile_pool(name="const", bufs=1) as cpool,
            tc.tile_pool(name="img", bufs=2) as ipool,
            tc.tile_pool(name="work", bufs=2) as wpool,
            tc.tile_pool(name="psum", bufs=2, space="PSUM") as ppool,
            tc.tile_pool(name="out", bufs=2) as opool,
        ):
            anc = cpool.tile([P, NANC * G], F32)
            nc.sync.dma_start(anc[:], anc_d[:])
            iden = cpool.tile([P, P], F32)
            nc.sync.dma_start(iden[:], iden_d[:])

            def ancp(k):
                return anc[:, k * G:(k + 1) * G]

            for b in [bb for _ in range(int(os.environ.get('DETLOSS_REPS', '1'))) for bb in range(BPC)]:
                ct = ipool.tile([P, C * G], F32, tag="ct", name="ct")
                nc.sync.dma_start(ct[:], cls_d[b])
                rt = ipool.tile([P, 4 * G], F32, tag="rt", name="rt")
                nc.sync.dma_start(rt[:], reg_d[b])
                gtt = ipool.tile([P, NGT * M], F32, tag="gtt", name="gtt")
                nc.sync.dma_start(gtt[:], gt_d[b])
                gtq = ipool.tile([P, 8], F32, tag="gtq", name="gtq")
                nc.sync.dma_start(gtq[0:64, :], gt5_d[b])
                # gathered per-GT quantities, chunk-major [8 x [128, 25*5]]
                pg5 = ipool.tile([P, NCHUNK * GPC * NQ], F32, tag="pg5", name="pg5")

                def g5(q):
                    """[128, 8, 25] strided view of gathered quantity q."""
                    a = pg5[:, :]
                    return bass.AP(a.tensor, a.offset + q,
                                   [a.ap[0], [GPC * NQ, NCHUNK], [NQ, GPC]])

                def clsp(k):
                    return ct[:, k * G:(k + 1) * G]

                def regp(k):
                    return rt[:, k * G:(k + 1) * G]

                def gtp(k):
                    return gtt[:, k * M:(k + 1) * M]

                def it(tag):
                    return ipool.tile([P, G], F32, tag=tag, name=tag)

                # ---- decode boxes ----
                cx = it("cx"); cy = it("cy"); w = it("w"); h = it("h")
                ew = it("ew"); hw = it("hw")
                x1 = it("x1"); x2 = it("x2"); y1 = it("y1"); y2 = it("y2")
                a1 = it("a1")
                nc.vector.tensor_tensor(cx[:], regp(0), ancp(A_WH), ALU.mult)
                nc.vector.tensor_tensor(cx[:], cx[:], ancp(A_CXM), ALU.add)
                nc.vector.tensor_tensor(cy[:], regp(1), ancp(A_HH), ALU.mult)
                nc.vector.tensor_tensor(cy[:], cy[:], ancp(A_CYM), ALU.add)
                nc.scalar.activation(ew[:], regp(2), ACT.Exp)
                nc.vector.tensor_tensor(w[:], ew[:], ancp(A_W), ALU.mult)
                nc.scalar.activation(ew[:], regp(3), ACT.Exp)
                nc.vector.tensor_tensor(h[:], ew[:], ancp(A_H), ALU.mult)
                nc.scalar.activation(hw[:], w[:], ACT.Copy, scale=0.5)
                nc.vector.tensor_sub(x1[:], cx[:], hw[:])
                nc.vector.tensor_add(x2[:], cx[:], hw[:])
                nc.scalar.activation(hw[:], h[:], ACT.Copy, scale=0.5)
                nc.vector.tensor_sub(y1[:], cy[:], hw[:])
                nc.vector.tensor_add(y2[:], cy[:], hw[:])
                nc.vector.tensor_mul(a1[:], w[:], h[:])

                # ---- pair stage: per-anchor max IoU + matched-GT gather ----
                mx = it("mx")
                gx1b = _rep_mid(gtp(G_X1), GPC)
                gy1b = _rep_mid(gtp(G_Y1), GPC)
                gx2b = _rep_mid(gtp(G_X2), GPC)
                gy2b = _rep_mid(gtp(G_Y2), GPC)
                a2b = _rep_mid(gtp(G_A2), GPC)

                for k in range(NCHUNK):
                    g0 = k * GPC
                    sl = slice(g0, g0 + GPC)

                    def wt(tag):
                        t = wpool.tile([P, GPC * M], F32, tag=tag, name=tag)
                        return t, t[:].rearrange("p (g m) -> p g m", m=M)

                    ta, tav = wt("ta"); tb, tbv = wt("tb"); tcn, tcv = wt("tc")
                    td, tdv = wt("td"); te, tev = wt("te"); tf, tfv = wt("tf")

                    nc.vector.tensor_tensor(tav, _rep_last(x1[:, sl], M), gx1b, ALU.max)
                    nc.vector.tensor_tensor(tbv, _rep_last(x2[:, sl], M), gx2b, ALU.min)
                    nc.gpsimd.tensor_tensor(tcv, tbv, tav, ALU.subtract)
                    nc.vector.tensor_tensor(tav, _rep_last(y1[:, sl], M), gy1b, ALU.max)
                    nc.vector.tensor_tensor(tbv, _rep_last(y2[:, sl], M), gy2b, ALU.min)
                    nc.gpsimd.tensor_tensor(tdv, tbv, tav, ALU.subtract)
                    nc.scalar.activation(ta[:], tcn[:], ACT.Relu)
                    nc.scalar.activation(tb[:], td[:], ACT.Relu)
                    nc.vector.tensor_mul(tcn[:], ta[:], tb[:])          # inter
                    nc.gpsimd.tensor_tensor(tdv, _rep_last(a1[:, sl], M), a2b, ALU.add)
                    nc.gpsimd.tensor_sub(te[:], td[:], tcn[:])          # union
                    nc.vector.reciprocal(tb[:], te[:])
                    nc.vector.tensor_mul(tf[:], tcn[:], tb[:])          # iou
                    nc.vector.reduce_max(mx[:, sl], tfv, axis=AX.X)
                    nc.vector.tensor_tensor(tav, tfv, _rep_last(mx[:, sl], M),
                                            ALU.is_equal)               # tie-mask
                    # PE gather: out[anchor, q] = sum_gt mask * gtq
                    pout = ppool.tile([P, GPC * NQ], F32, tag="pout", name="pout")
                    for g in range(GPC):
                        w0 = g * M
                        psT = ppool.tile([M, P], F32, tag="psT", name="psT")
                        nc.tensor.transpose(psT[:], ta[:, w0:w0 + M], iden[:])
                        ohT = wpool.tile([M, P], F32, tag="ohT", name="ohT")
                        nc.scalar.activation(ohT[:], psT[:], ACT.Copy)
                        nc.tensor.matmul(pout[:, g * NQ:(g + 1) * NQ],
                                         ohT[:], gtq[0:M, 0:NQ])
                    nc.scalar.activation(pg5[:, k * GPC * NQ:(k + 1) * GPC * NQ],
                                         pout[:], ACT.Copy)

                # ---- classification ----
                acc = it("acc"); tmp = it("tmp"); tmp2 = it("tmp2")
                lse = it("lse"); bgt = it("bgt"); xt = it("xt"); tgtt = it("tgtt")
                posf = it("posf"); negf = it("negf"); scr = it("scr")
                for c in range(C):
                    if c == 0:
                        nc.scalar.activation(acc[:], clsp(0), ACT.Exp)
                    else:
                        nc.scalar.activation(tmp2[:], clsp(c), ACT.Exp)
                        nc.gpsimd.tensor_add(acc[:], acc[:], tmp2[:])
                nc.scalar.activation(lse[:], acc[:], ACT.Ln)
                nc.vector.tensor_sub(bgt[:], lse[:], clsp(0))           # ce_bg
                labv = g5(4)
                for c in range(C):
                    if c == 0:
                        nc.vector.scalar_tensor_tensor(
                            _v3(xt[:]), labv, 0.0, _v3(clsp(0)), ALU.is_equal, ALU.mult)
                    else:
                        nc.vector.scalar_tensor_tensor(
                            _v3(tmp[:]), labv, float(c), _v3(clsp(c)),
                            ALU.is_equal, ALU.mult)
                        nc.vector.tensor_add(xt[:], xt[:], tmp[:])
                nc.vector.tensor_sub(tgtt[:], lse[:], xt[:])            # ce_tgt
                nc.vector.tensor_scalar(posf[:], mx[:], 0.25, None, ALU.is_ge)
                nc.vector.tensor_scalar(negf[:], mx[:], 0.1, None, ALU.is_lt)

                ot = opool.tile([P, NOUT], F32, tag="ot", name="ot")
                nc.scalar.activation(scr[:], posf[:], ACT.Copy, accum_out=ot[:, 0:1])
                nc.scalar.activation(scr[:], negf[:], ACT.Copy, accum_out=ot[:, 1:2])
                nc.scalar.activation(scr[:], bgt[:], ACT.Copy, accum_out=ot[:, 2:3])
                nc.vector.tensor_mul(scr[:], tgtt[:], posf[:])
                nc.vector.reduce_sum(ot[:, 3:4],
                                     scr[:].rearrange("p (g m) -> p g m", g=1), axis=AX.X)
                nc.vector.tensor_mul(scr[:], bgt[:], negf[:])
                nc.vector.reduce_sum(ot[:, 4:5],
                                     scr[:].rearrange("p (g m) -> p g m", g=1), axis=AX.X)

                # ---- regression smooth-L1 (negated sums) ----
                dd = it("dd"); nsl = it("nsl"); za = it("za")

                def huber_neg(first, d):
                    nc.scalar.activation(tmp[:], d, ACT.Abs)
                    nc.vector.tensor_scalar(tmp2[:], tmp[:], 1.0, None, ALU.min)  # z
                    nc.vector.tensor_mul(za[:], tmp2[:], tmp[:])                  # z*a
                    nc.vector.tensor_mul(tmp2[:], tmp2[:], tmp2[:])               # z^2
                    nc.vector.scalar_tensor_tensor(
                        tmp[:], tmp2[:], 0.5, za[:], ALU.mult, ALU.subtract)      # .5z^2-za
                    if first:
                        nc.vector.tensor_copy(nsl[:], tmp[:])
                    else:
                        nc.gpsimd.tensor_add(nsl[:], nsl[:], tmp[:])

                for comp, (q, acp, invp) in enumerate(
                        ((0, A_CX, A_I2W), (1, A_CY, A_I2H))):
                    nc.vector.tensor_tensor(_v3(tmp[:]), g5(q), _v3(ancp(acp)),
                                            ALU.subtract)
                    nc.vector.tensor_tensor(tmp[:], tmp[:], ancp(invp), ALU.mult)
                    nc.vector.tensor_scalar(tmp2[:], regp(comp), 0.5, None, ALU.subtract)
                    nc.vector.tensor_sub(dd[:], tmp2[:], tmp[:])
                    huber_neg(comp == 0, dd[:])
                for comp, (q, lgp) in enumerate(((2, A_LW), (3, A_LH))):
                    nc.vector.tensor_tensor(_v3(tmp[:]), g5(q), _v3(ancp(lgp)),
                                            ALU.subtract)
                    nc.vector.tensor_sub(dd[:], regp(comp + 2), tmp[:])
                    huber_neg(False, dd[:])
                nc.vector.tensor_mul(scr[:], nsl[:], posf[:])
                nc.vector.reduce_sum(ot[:, 5:6],
                                     scr[:].rearrange("p (g m) -> p g m", g=1), axis=AX.X)

                nc.sync.dma_start(res_d[b], ot[:])
    nc.compile()
    return nc


_NC_CACHE = None


def _get_nc():
    global _NC_CACHE
    if _NC_CACHE is None:
        _NC_CACHE = build_program()
    return _NC_CACHE


def prep_inputs(cls_output, reg_output, anchors, gt_boxes, gt_labels, num_boxes):
    """Host-side shard + derived-plane prep. Returns (in_maps, num_boxes)."""
    B = cls_output.shape[0]
    cls_output = np.asarray(cls_output, np.float32)
    reg_output = np.asarray(reg_output, np.float32)
    anchors = np.asarray(anchors, np.float32)
    gt_boxes = np.asarray(gt_boxes, np.float32)
    gt_labels = np.asarray(gt_labels)
    num_boxes = np.asarray(num_boxes)

    aw = anchors[:, 2] - anchors[:, 0]
    ah = anchors[:, 3] - anchors[:, 1]
    acx = anchors[:, 0] + 0.5 * aw
    acy = anchors[:, 1] + 0.5 * ah
    planes = np.stack([
        acx - aw / 4.0, acy - ah / 4.0, aw / 2.0, ah / 2.0, aw, ah,
        2.0 / aw, 2.0 / ah, np.log(aw), np.log(ah), acx, acy,
    ], axis=0).astype(np.float32)                       # [12, N]
    anc_host = planes.reshape(NANC, P, G).transpose(1, 0, 2).reshape(P, NANC * G)

    gx1 = gt_boxes[..., 0]; gy1 = gt_boxes[..., 1]
    gx2 = gt_boxes[..., 2]; gy2 = gt_boxes[..., 3]
    area2 = (gx2 - gx1) * (gy2 - gy1)
    valid = np.arange(M)[None, :] < num_boxes[:, None]
    area2 = np.where(valid, area2, np.float32(1e30)).astype(np.float32)
    gt_all = np.stack([gx1, gy1, gx2, gy2, area2], axis=1)        # [B,5,M]
    gt_host = np.broadcast_to(gt_all[:, None, :, :], (B, P, NGT, M)) \
        .reshape(B, P, NGT * M).astype(np.float32)

    gw = gx2 - gx1; gh = gy2 - gy1
    gcx = gx1 + np.float32(0.5) * gw
    gcy = gy1 + np.float32(0.5) * gh
    lgw = np.log(np.maximum(gw, np.float32(1e-6)))
    lgh = np.log(np.maximum(gh, np.float32(1e-6)))
    gt5_host = np.zeros((B, 64, 8), np.float32)
    gt5_host[:, :M, 0] = gcx; gt5_host[:, :M, 1] = gcy
    gt5_host[:, :M, 2] = lgw; gt5_host[:, :M, 3] = lgh
    gt5_host[:, :M, 4] = gt_labels.astype(np.float32)

    cls_host = cls_output.reshape(B, C, P, G).transpose(0, 2, 1, 3) \
        .reshape(B, P, C * G).copy()
    reg_host = reg_output.reshape(B, 4, P, G).transpose(0, 2, 1, 3) \
        .reshape(B, P, 4 * G).copy()
    iden = np.eye(P, dtype=np.float32)

    in_maps = []
    for core in range(NCORES):
        s = slice(core * BPC, (core + 1) * BPC)
        in_maps.append({
            "cls": np.ascontiguousarray(cls_host[s]),
            "reg": np.ascontiguousarray(reg_host[s]),
            "anc": anc_host,
            "gt": np.ascontiguousarray(gt_host[s]),
            "gt5": np.ascontiguousarray(gt5_host[s]),
            "iden": iden,
        })
    return in_maps, num_boxes


def finish(res_all, num_boxes):
    """res_all: [B, P, NOUT] partial sums. Reproduce reference scalar combine."""
    s = res_all.sum(axis=1).astype(np.float32)          # [B, NOUT]
    npos, nneg, ce_bg_sum, ce_tgt_pos, ce_bg_neg, neg_sl = (s[:, i] for i in range(6))
    sl_pos = -neg_sl
    has = num_boxes > 0
    cls_pos = np.where(npos > 0, ce_tgt_pos / np.maximum(npos, 1.0), 0.0)
    cls_neg = np.where(nneg > 0, ce_bg_neg / np.maximum(nneg, 1.0), 0.0)
    cls_losses = np.where(has, cls_pos + cls_neg, ce_bg_sum / np.float32(N))
    reg_losses = np.where(npos > 0, sl_pos / np.maximum(npos * 4.0, 1.0), 0.0)
    total_pos = npos.sum(dtype=np.float32)
    cls_final = np.float32(cls_losses.astype(np.float32).mean())
    reg_final = np.float32(reg_losses.astype(np.float32).sum() / max(total_pos, 1.0))
    total = np.float32(cls_final + reg_final)
    return total, cls_final, reg_final, np.float32(total_pos)


def kernel(cls_output, reg_output, anchors, gt_boxes, gt_labels, num_boxes):
    nc = _get_nc()
    in_maps, num_boxes = prep_inputs(
        cls_output, reg_output, anchors, gt_boxes, gt_labels, num_boxes)
    out = run_bass_kernel_spmd(nc, in_maps, list(range(NCORES)))
    res_all = np.concatenate([np.asarray(r["res"]) for r in out.results], axis=0)
    return finish(res_all, num_boxes)



# revision 12
# speedup vs baseline: 1.2922x; 1.2922x over previous
"""DetectionLoss Trainium2 kernel v2: 8-core data-parallel (4 images/core).

Per image the device computes partial sums over anchors ([128,6]):
  [npos, nneg, sum(ce_bg), sum(ce_tgt*posf), sum(ce_bg*negf), -sum(sl*posf)]
and the host finishes the scalar combine exactly as the reference does.

Key design vs the old PE-gather version:
- GT matching runs in blocks of 16 GTs (block count JIT-compiled per input
  num_boxes, images greedily rebalanced across cores).
- Per-pair score z = quant64(K * inter * recip_approx(s)) + m packs the GT
  index into the score; ONE reduce_max gives both thresholds (compared on
  zmax directly) and the matched-GT one-hot (is_eq vs zmax).
- Gather of matched [gcx, gcy, lgw, lgh, label] runs as fp16 one-hot mask ->
  DMA transpose (XBAR) -> one fp16 matmul per 8 anchor groups. No PE
  transposes, no per-group stationary reloads, no PSUM->SBUF mask copies.
"""
import sys
import numpy as np

sys.path.insert(0, "/opt/trn_rl_repo")

import concourse.bass as bass
import concourse.bacc as bacc
import concourse.mybir as mybir
from concourse import tile
from concourse.bass_utils import run_bass_kernel_spmd
from concourse.dve_ops import RECIPROCAL_APPROX_FAST, RECIP_APPROX_FAST_CONSTS

F32 = mybir.dt.float32
F16 = mybir.dt.float16
ALU = mybir.AluOpType
ACT = mybir.ActivationFunctionType
AX = mybir.AxisListType

P = 128          # partitions
G = 200          # anchor groups per partition (N = P*G = 25600)
N = P * G
C = 8            # classes
BPC = 4          # images per core
NCORES = 8
MBLK = 16        # GTs per block
MAXBLK = 4       # max blocks (<= 64 GTs total for m-packing)
NQ = 5           # gathered quantities
NOUT = 6

KPACK = float(2 ** 22)
C29 = float(2 ** 29)
TPOS = KPACK * 0.2 + 31.5
TNEG = KPACK / 11.0 + 31.5

# chunking of the 200 groups: units of 8 groups (128 mask cols) per matmul
CHUNKS = ((0, 72), (72, 64), (136, 64))   # (group base, group count)

# anchor plane indices
A_CXM, A_CYM, A_WH, A_HH, A_W, A_H, A_I2W, A_I2H, A_PX, A_PY, A_LW, A_LH = range(12)
NANC = 12
# gt block plane indices (each MBLK wide)
G_X1, G_Y1, G_X2, G_Y2, G_A2K = range(5)
NGT = 5


def _rep_last(ap, n):
    """[..., d] -> [..., d, n] with step-0 broadcast."""
    return bass.AP(ap.tensor, ap.offset, list(ap.ap) + [[0, n]])


def _rep_mid(ap, n):
    """[p, d] -> [p, n, d] with step-0 broadcast."""
    a = list(ap.ap)
    return bass.AP(ap.tensor, ap.offset, [a[0], [0, n]] + a[1:])


def _stride5(ap2d, q):
    """[128, 1000] cand tile -> [128, 200] plane of quantity q."""
    a = ap2d
    return bass.AP(a.tensor, a.offset + q, [a.ap[0], [NQ, G]])


def build_program(slot_blocks):
    """slot_blocks: tuple of BPC ints (blocks per image slot, each <= MAXBLK)."""
    nc = bacc.Bacc(None, target_bir_lowering=False)
    cls_d = nc.dram_tensor("cls", [BPC, P, C * G], F32, kind="ExternalInput")
    reg_d = nc.dram_tensor("reg", [BPC, P, 4 * G], F32, kind="ExternalInput")
    anc_d = nc.dram_tensor("anc", [P, NANC * G], F32, kind="ExternalInput")
    gtp_d = nc.dram_tensor("gtp", [BPC, MAXBLK, P, NGT * MBLK], F32,
                           kind="ExternalInput")
    gtq_d = nc.dram_tensor("gtq", [BPC, MAXBLK, P, 8 * NQ], F16,
                           kind="ExternalInput")
    iota_d = nc.dram_tensor("iota", [P, MAXBLK * MBLK], F32, kind="ExternalInput")
    res_d = nc.dram_tensor("res", [BPC, P, NOUT], F32, kind="ExternalOutput")

    rk = RECIP_APPROX_FAST_CONSTS

    with tile.TileContext(nc) as tc:
        with (
            tc.tile_pool(name="const", bufs=1) as cpool,
            tc.tile_pool(name="img", bufs=2) as ipool,
            tc.tile_pool(name="blk", bufs=2) as bpool,
            tc.tile_pool(name="chk", bufs=2) as kpool,
            tc.tile_pool(name="psum", bufs=2, space="PSUM") as ppool,
            tc.tile_pool(name="out", bufs=2) as opool,
        ):
            anc = cpool.tile([P, NANC * G], F32)
            nc.sync.dma_start(anc[:], anc_d[:])
            iota = cpool.tile([P, MAXBLK * MBLK], F32)
            nc.sync.dma_start(iota[:], iota_d[:])

            def ancp(k):
                return anc[:, k * G:(k + 1) * G]

            for b in range(BPC):
                nblk = slot_blocks[b]
                ct = ipool.tile([P, C * G], F32, tag="ct", name="ct")
                nc.sync.dma_start(ct[:], cls_d[b])
                rt = ipool.tile([P, 4 * G], F32, tag="rt", name="rt")
                nc.sync.dma_start(rt[:], reg_d[b])

                def it(tag):
                    return ipool.tile([P, G], F32, tag=tag, name=tag)

                def clsp(k):
                    return ct[:, k * G:(k + 1) * G]

                def regp(k):
                    return rt[:, k * G:(k + 1) * G]

                # ---- decode ----
                cx = it("cx"); cy = it("cy"); w = it("w"); h = it("h")
                ew = it("ew")
                x1 = it("x1"); x2 = it("x2"); y1 = it("y1"); y2 = it("y2")
                a1 = it("a1")
                nc.vector.tensor_tensor(cx[:], regp(0), ancp(A_WH), ALU.mult)
                nc.gpsimd.tensor_tensor(cx[:], cx[:], ancp(A_CXM), ALU.add)
                nc.vector.tensor_tensor(cy[:], regp(1), ancp(A_HH), ALU.mult)
                nc.gpsimd.tensor_tensor(cy[:], cy[:], ancp(A_CYM), ALU.add)
                nc.scalar.activation(ew[:], regp(2), ACT.Exp)
                nc.vector.tensor_tensor(w[:], ew[:], ancp(A_W), ALU.mult)
                nc.scalar.activation(ew[:], regp(3), ACT.Exp)
                nc.vector.tensor_tensor(h[:], ew[:], ancp(A_H), ALU.mult)
                nc.vector.scalar_tensor_tensor(x1[:], w[:], -0.5, cx[:],
                                               ALU.mult, ALU.add)
                nc.vector.scalar_tensor_tensor(x2[:], w[:], 0.5, cx[:],
                                               ALU.mult, ALU.add)
                nc.vector.scalar_tensor_tensor(y1[:], h[:], -0.5, cy[:],
                                               ALU.mult, ALU.add)
                nc.vector.scalar_tensor_tensor(y2[:], h[:], 0.5, cy[:],
                                               ALU.mult, ALU.add)
                nc.gpsimd.tensor_tensor(a1[:], w[:], h[:], ALU.mult)
                a1K = it("a1K")
                nc.vector.tensor_scalar(a1K[:], a1[:], 2.0 ** -22, None,
                                        ALU.mult)

                # reg-target precombines: regNP = reg_n + plane
                r0p = it("r0p"); r1p = it("r1p"); r2p = it("r2p"); r3p = it("r3p")
                nc.gpsimd.tensor_tensor(r0p[:], regp(0), ancp(A_PX), ALU.add)
                nc.gpsimd.tensor_tensor(r1p[:], regp(1), ancp(A_PY), ALU.add)
                nc.gpsimd.tensor_tensor(r2p[:], regp(2), ancp(A_LW), ALU.add)
                nc.gpsimd.tensor_tensor(r3p[:], regp(3), ancp(A_LH), ALU.add)

                zmax_run = ipool.tile([P, G], F32, tag="zmax_run", name="zmax_run")
                cand_run = ipool.tile([P, NQ * G], F32, tag="cand_run",
                                      name="cand_run")
                if nblk == 0:
                    nc.gpsimd.memset(zmax_run[:], 0.0)
                    nc.gpsimd.memset(cand_run[:], 0.0)

                # ---- GT blocks ----
                for blk in range(nblk):
                    gtt = bpool.tile([P, NGT * MBLK], F32, tag="gtt", name="gtt")
                    nc.sync.dma_start(gtt[:], gtp_d[b, blk])
                    gtqt = bpool.tile([P, 8 * NQ], F16, tag="gtqt", name="gtqt")
                    nc.sync.dma_start(gtqt[:], gtq_d[b, blk])
                    if blk == 0:
                        zmax_blk, cand_blk = zmax_run, cand_run
                    else:
                        zmax_blk = bpool.tile([P, G], F32, tag="zmax_blk",
                                              name="zmax_blk")
                        cand_blk = bpool.tile([P, NQ * G], F32, tag="cand_blk",
                                              name="cand_blk")

                    def gtp(k):
                        return gtt[:, k * MBLK:(k + 1) * MBLK]

                    for (g0, gc) in CHUNKS:
                        cols = gc * MBLK
                        units = cols // 128
                        sl = slice(g0, g0 + gc)

                        def kt(tag):
                            return kpool.tile([P, CHUNKS[0][1] * MBLK], F32,
                                              tag=tag, name=tag)

                        tA = kt("tA"); tB = kt("tB"); tC = kt("tC")
                        tD = kt("tD")
                        mask = kpool.tile([P, CHUNKS[0][1] * MBLK], F16,
                                          tag="mask", name="mask")
                        maskT = kpool.tile([P, CHUNKS[0][1] * MBLK], F16,
                                           tag="maskT", name="maskT")
                        vA = tA[:, 0:cols].rearrange("p (g m) -> p g m", m=MBLK)
                        vB = tB[:, 0:cols].rearrange("p (g m) -> p g m", m=MBLK)
                        vC = tC[:, 0:cols].rearrange("p (g m) -> p g m", m=MBLK)
                        vD = tD[:, 0:cols].rearrange("p (g m) -> p g m", m=MBLK)

                        # pair stage
                        nc.vector.tensor_tensor(vA, _rep_last(x1[:, sl], MBLK),
                                                _rep_mid(gtp(G_X1), gc), ALU.max)
                        nc.vector.tensor_tensor(vB, _rep_last(x2[:, sl], MBLK),
                                                _rep_mid(gtp(G_X2), gc), ALU.min)
                        nc.gpsimd.tensor_tensor(vC, vB, vA, ALU.subtract)  # iwr
                        nc.vector.tensor_tensor(vA, _rep_last(y1[:, sl], MBLK),
                                                _rep_mid(gtp(G_Y1), gc), ALU.max)
                        nc.vector.tensor_tensor(vB, _rep_last(y2[:, sl], MBLK),
                                                _rep_mid(gtp(G_Y2), gc), ALU.min)
                        nc.gpsimd.tensor_tensor(vD, vB, vA, ALU.subtract)  # ihr
                        nc.scalar.activation(tB[:, 0:cols], tD[:, 0:cols],
                                             ACT.Relu)                     # ihp
                        nc.scalar.activation(tA[:, 0:cols], tC[:, 0:cols],
                                             ACT.Relu)                     # iwp
                        nc.gpsimd.tensor_tensor(vC, vA, vB, ALU.mult)      # inter
                        nc.gpsimd.tensor_tensor(
                            vA, _rep_last(a1K[:, sl], MBLK),
                            _rep_mid(gtp(G_A2K), gc), ALU.add)             # s'
                        nc.vector._custom_dve(RECIPROCAL_APPROX_FAST,
                                              out=tD[:, 0:cols], in0=tA[:, 0:cols],
                                              s0=rk["s0"], s1=rk["s1"],
                                              imm2=rk["imm2"])             # rcp
                        nc.gpsimd.tensor_tensor(vB, vC, vD, ALU.mult)      # zK
                        nc.scalar.activation(tA[:, 0:cols], tB[:, 0:cols],
                                             ACT.Copy, bias=C29)           # q1
                        nc.scalar.activation(tD[:, 0:cols], tA[:, 0:cols],
                                             ACT.Copy, bias=-C29)          # q2
                        nc.gpsimd.tensor_tensor(
                            vA, vD,
                            _rep_mid(iota[:, blk * MBLK:(blk + 1) * MBLK], gc),
                            ALU.add)                                       # z
                        nc.vector.reduce_max(zmax_blk[:, sl], vA, axis=AX.X)
                        nc.vector.tensor_tensor(
                            mask[:, 0:cols].rearrange("p (g m) -> p g m", m=MBLK),
                            vA, _rep_last(zmax_blk[:, sl], MBLK), ALU.is_equal)

                        # gather: DMA transpose + fp16 matmuls
                        ps = ppool.tile([P, NQ * CHUNKS[0][1]], F32, tag="ps",
                                        name="ps")
                        for u in range(units):
                            usl = slice(u * 128, (u + 1) * 128)
                            nc.sync.dma_start_transpose(maskT[:, usl],
                                                        mask[:, usl])
                            nc.tensor.matmul(ps[:, u * 40:(u + 1) * 40],
                                             maskT[:, usl], gtqt[:])
                        nc.scalar.activation(
                            cand_blk[:, NQ * g0:NQ * (g0 + gc)],
                            ps[:, 0:NQ * gc], ACT.Copy)

                    if blk > 0:
                        bsel5f = bpool.tile([P, NQ * G], F32, tag="bsel5f",
                                            name="bsel5f")
                        nc.vector.tensor_tensor(
                            bsel5f[:].rearrange("p (g q) -> p g q", q=NQ),
                            _rep_last(zmax_blk[:], NQ),
                            _rep_last(zmax_run[:], NQ), ALU.is_gt)
                        bsel5 = bpool.tile([P, NQ * G], mybir.dt.uint8,
                                           tag="bsel5", name="bsel5")
                        nc.vector.tensor_copy(bsel5[:], bsel5f[:])
                        nc.vector.copy_predicated(cand_run[:], bsel5[:],
                                                  cand_blk[:])
                        nc.vector.tensor_tensor(zmax_run[:], zmax_run[:],
                                                zmax_blk[:], ALU.max)

                # ---- thresholds ----
                posf = it("posf"); negf = it("negf")
                nc.vector.tensor_scalar(posf[:], zmax_run[:], TPOS, None,
                                        ALU.is_ge)
                nc.vector.tensor_scalar(negf[:], zmax_run[:], TNEG, None,
                                        ALU.is_lt)

                # ---- classification ----
                ext = ipool.tile([P, C * G], F32, tag="ext", name="ext")
                nc.scalar.activation(ext[:], ct[:], ACT.Exp)
                sumex = it("sumex"); lse = it("lse"); bgt = it("bgt")
                xt = it("xt"); tmp = it("tmp"); tmp2 = it("tmp2")
                nc.vector.reduce_sum(
                    sumex[:],
                    bass.AP(ext[:].tensor, ext[:].offset,
                            [ext[:].ap[0], [1, G], [G, C]]),
                    axis=AX.X)
                nc.scalar.activation(lse[:], sumex[:], ACT.Ln)
                nc.gpsimd.tensor_tensor(bgt[:], lse[:], clsp(0), ALU.subtract)
                lab = _stride5(cand_run[:], 4)
                for c in range(C):
                    if c == 0:
                        nc.vector.scalar_tensor_tensor(xt[:], lab, 0.0, clsp(0),
                                                       ALU.is_equal, ALU.mult)
                    else:
                        nc.vector.scalar_tensor_tensor(tmp[:], lab, float(c),
                                                       clsp(c), ALU.is_equal,
                                                       ALU.mult)
                        nc.gpsimd.tensor_tensor(xt[:], xt[:], tmp[:], ALU.add)
                tgtt = it("tgtt")
                nc.gpsimd.tensor_tensor(tgtt[:], lse[:], xt[:], ALU.subtract)

                ot = opool.tile([P, NOUT], F32, tag="ot", name="ot")
                scr = it("scr")
                nc.scalar.activation(scr[:], posf[:], ACT.Copy,
                                     accum_out=ot[:, 0:1])
                nc.scalar.activation(scr[:], negf[:], ACT.Copy,
                                     accum_out=ot[:, 1:2])
                nc.scalar.activation(scr[:], bgt[:], ACT.Copy,
                                     accum_out=ot[:, 2:3])
                nc.vector.tensor_tensor(scr[:], tgtt[:], posf[:], ALU.mult)
                nc.scalar.activation(tmp2[:], scr[:], ACT.Copy,
                                     accum_out=ot[:, 3:4])
                nc.vector.tensor_tensor(scr[:], bgt[:], negf[:], ALU.mult)
                nc.scalar.activation(tmp2[:], scr[:], ACT.Copy,
                                     accum_out=ot[:, 4:5])

                # ---- regression smooth-L1 ----
                nsl = it("nsl"); dd = it("dd"); ad = it("ad"); zc = it("zc")
                for comp, (rp, invp) in enumerate(
                        ((r0p, A_I2W), (r1p, A_I2H), (r2p, None), (r3p, None))):
                    qsel = _stride5(cand_run[:], comp)
                    if invp is not None:
                        nc.vector.tensor_tensor(dd[:], qsel, ancp(invp), ALU.mult)
                        nc.gpsimd.tensor_tensor(dd[:], rp[:], dd[:], ALU.subtract)
                    else:
                        nc.gpsimd.tensor_tensor(dd[:], rp[:], qsel, ALU.subtract)
                    nc.scalar.activation(ad[:], dd[:], ACT.Abs)
                    nc.vector.tensor_scalar(zc[:], ad[:], 1.0, None, ALU.min)
                    nc.vector.scalar_tensor_tensor(dd[:], zc[:], -0.5, ad[:],
                                                   ALU.mult, ALU.add)  # |d|-z/2
                    if comp == 0:
                        nc.gpsimd.tensor_tensor(nsl[:], zc[:], dd[:], ALU.mult)
                    else:
                        nc.vector.tensor_tensor(tmp[:], zc[:], dd[:], ALU.mult)
                        nc.gpsimd.tensor_tensor(nsl[:], nsl[:], tmp[:], ALU.add)
                nc.vector.tensor_tensor(scr[:], nsl[:], posf[:], ALU.mult)
                nc.scalar.activation(tmp2[:], scr[:], ACT.Copy, scale=-1.0,
                                     accum_out=ot[:, 5:6])

                nc.sync.dma_start(res_d[b], ot[:])
    nc.compile()
    return nc


# fix the is_eq->mask emission above: write into mask tile (f16)
# (rearrange helper path kept simple: the call writes mask directly)


_NC_CACHE = {}


def _get_nc(slot_blocks):
    key = tuple(slot_blocks)
    if key not in _NC_CACHE:
        _NC_CACHE[key] = build_program(key)
    return _NC_CACHE[key]


def plan_assignment(num_boxes):
    """Greedy round-robin of images (desc by block count) across cores.
    Returns (order [32] image indices per (core, slot)), slot_blocks."""
    nb = np.asarray(num_boxes)
    blocks = np.ceil(nb / MBLK).astype(int)
    order = np.argsort(-blocks, kind="stable")
    assign = np.zeros((NCORES, BPC), int)
    for rank, img in enumerate(order):
        core = rank % NCORES
        slot = rank // NCORES
        assign[core, slot] = img
    slot_blocks = tuple(
        int(max(blocks[assign[c, s]] for c in range(NCORES)))
        for s in range(BPC))
    return assign, slot_blocks


def prep_inputs(cls_output, reg_output, anchors, gt_boxes, gt_labels, num_boxes):
    B = cls_output.shape[0]
    cls_output = np.asarray(cls_output, np.float32)
    reg_output = np.asarray(reg_output, np.float32)
    anchors = np.asarray(anchors, np.float32)
    gt_boxes = np.asarray(gt_boxes, np.float32)
    gt_labels = np.asarray(gt_labels)
    num_boxes = np.asarray(num_boxes)

    aw = anchors[:, 2] - anchors[:, 0]
    ah = anchors[:, 3] - anchors[:, 1]
    acx = anchors[:, 0] + 0.5 * aw
    acy = anchors[:, 1] + 0.5 * ah
    i2w = 2.0 / aw
    i2h = 2.0 / ah
    planes = np.stack([
        acx - aw / 4.0, acy - ah / 4.0, aw / 2.0, ah / 2.0, aw, ah,
        i2w, i2h, acx * i2w - 0.5, acy * i2h - 0.5, np.log(aw), np.log(ah),
    ], axis=0).astype(np.float32)                       # [12, N]
    anc_host = planes.reshape(NANC, P, G).transpose(1, 0, 2).reshape(P, NANC * G)

    assign, slot_blocks = plan_assignment(num_boxes)

    cls_host = cls_output.reshape(B, C, P, G).transpose(0, 2, 1, 3) \
        .reshape(B, P, C * G)
    reg_host = reg_output.reshape(B, 4, P, G).transpose(0, 2, 1, 3) \
        .reshape(B, P, 4 * G)

    iota_host = np.broadcast_to(
        np.arange(MAXBLK * MBLK, dtype=np.float32)[None, :],
        (P, MAXBLK * MBLK)).copy()

    # per image: gt block planes + fp16 blockdiag q tables
    gw = gt_boxes[..., 2] - gt_boxes[..., 0]
    gh = gt_boxes[..., 3] - gt_boxes[..., 1]
    gcx = gt_boxes[..., 0] + 0.5 * gw
    gcy = gt_boxes[..., 1] + 0.5 * gh
    lgw = np.log(np.maximum(gw, 1e-6))
    lgh = np.log(np.maximum(gh, 1e-6))
    area = (gw * gh).astype(np.float32)

    def image_blocks(img):
        nbx = int(num_boxes[img])
        gtp = np.zeros((MAXBLK, NGT, MBLK), np.float32)
        q5 = np.zeros((MAXBLK, MBLK, NQ), np.float16)
        for m in range(min(nbx, 50)):
            blk, mm = divmod(m, MBLK)
            gtp[blk, G_X1, mm] = gt_boxes[img, m, 0]
            gtp[blk, G_Y1, mm] = gt_boxes[img, m, 1]
            gtp[blk, G_X2, mm] = gt_boxes[img, m, 2]
            gtp[blk, G_Y2, mm] = gt_boxes[img, m, 3]
            gtp[blk, G_A2K, mm] = area[img, m] * np.float32(2.0 ** -22)
            q5[blk, mm] = np.float16([gcx[img, m], gcy[img, m], lgw[img, m],
                                      lgh[img, m], float(gt_labels[img, m])])
        # blockdiag [128, 40]: rows 16g'+m, cols 5g'+q
        gtq = np.zeros((MAXBLK, P, 8 * NQ), np.float16)
        for gp in range(8):
            gtq[:, gp * MBLK:(gp + 1) * MBLK, gp * NQ:(gp + 1) * NQ] = q5
        gtp_flat = np.broadcast_to(
            gtp.reshape(MAXBLK, 1, NGT * MBLK), (MAXBLK, P, NGT * MBLK))
        return gtp_flat.astype(np.float32), gtq

    in_maps = []
    for core in range(NCORES):
        imgs = [int(assign[core, s]) for s in range(BPC)]
        gtp_c = np.zeros((BPC, MAXBLK, P, NGT * MBLK), np.float32)
        gtq_c = np.zeros((BPC, MAXBLK, P, 8 * NQ), np.float16)
        for s, img in enumerate(imgs):
            gtp_c[s], gtq_c[s] = image_blocks(img)
        in_maps.append({
            "cls": np.ascontiguousarray(cls_host[imgs]),
            "reg": np.ascontiguousarray(reg_host[imgs]),
            "anc": anc_host,
            "gtp": gtp_c,
            "gtq": gtq_c,
            "iota": iota_host,
        })
    return in_maps, num_boxes, assign, slot_blocks


def finish(res_all, num_boxes):
    """res_all: [B, P, NOUT] partial sums in ORIGINAL image order."""
    s = res_all.sum(axis=1).astype(np.float32)          # [B, NOUT]
    npos, nneg, ce_bg_sum, ce_tgt_pos, ce_bg_neg, neg_sl = (s[:, i]
                                                            for i in range(6))
    sl_pos = -neg_sl
    has = num_boxes > 0
    cls_pos = np.where(npos > 0, ce_tgt_pos / np.maximum(npos, 1.0), 0.0)
    cls_neg = np.where(nneg > 0, ce_bg_neg / np.maximum(nneg, 1.0), 0.0)
    cls_losses = np.where(has, cls_pos + cls_neg, ce_bg_sum / np.float32(N))
    reg_losses = np.where(npos > 0, sl_pos / np.maximum(npos * 4.0, 1.0), 0.0)
    total_pos = npos.sum(dtype=np.float32)
    cls_final = np.float32(cls_losses.astype(np.float32).mean())
    reg_final = np.float32(reg_losses.astype(np.float32).sum()
                           / max(total_pos, 1.0))
    total = np.float32(cls_final + reg_final)
    return total, cls_final, reg_final, np.float32(total_pos)


def kernel(cls_output, reg_output, anchors, gt_boxes, gt_labels, num_boxes):
    in_maps, num_boxes, assign, slot_blocks = prep_inputs(
        cls_output, reg_output, anchors, gt_boxes, gt_labels, num_boxes)
    nc = _get_nc(slot_blocks)
    out = run_bass_kernel_spmd(nc, in_maps, list(range(NCORES)))
    res_perm = np.concatenate([np.asarray(r["res"]) for r in out.results],
                              axis=0)                   # [32, P, 6] core order
    res_all = np.zeros_like(res_perm)
    flat_assign = assign.reshape(-1)
    for pos, img in enumerate(flat_assign):
        res_all[img] = res_perm[pos]
    return finish(res_all, np.asarray(num_boxes))


# revision 17
# speedup vs baseline: 1.6943x; 1.3111x over previous
"""DetectionLoss Trainium2 kernel v3: 8-core data-parallel (4 images/core).

Per image the device computes partial sums over anchors ([128,6]):
  [npos, nneg, sum(ce_bg), sum(ce_tgt*posf), sum(ce_bg*negf), -sum(sl*posf)]
and the host finishes the scalar combine exactly as the reference does.

Design:
- GT matching in blocks of 16 GTs (block count JIT-compiled per input
  num_boxes; images greedily rebalanced across cores).
- Per-pair score z = quant64(K * inter * recip_approx(s)) + m packs the GT
  index into the score; ONE reduce_max per chunk gives thresholds (compared
  on zmax directly) and the matched-GT one-hot (is_eq vs zmax).
- Gather of matched [gcx, gcy, lgw, lgh, label] as fp16 one-hot mask ->
  batched XBAR DMA transpose (1 instr/chunk) -> one fp16 matmul per
  8 anchor groups.
- Tail math packed into wide multi-plane ops (decode, xt, huber) to
  amortize the ~1us fixed cost per HW instruction.
"""
import sys
import numpy as np

sys.path.insert(0, "/opt/trn_rl_repo")

import concourse.bass as bass
import concourse.bacc as bacc
import concourse.mybir as mybir
from concourse import tile
from concourse.bass_utils import run_bass_kernel_spmd
from concourse.dve_ops import RECIPROCAL_APPROX_FAST, RECIP_APPROX_FAST_CONSTS

F32 = mybir.dt.float32
F16 = mybir.dt.float16
U8 = mybir.dt.uint8
ALU = mybir.AluOpType
ACT = mybir.ActivationFunctionType
AX = mybir.AxisListType

P = 128          # partitions
G = 200          # anchor groups per partition (N = P*G = 25600)
N = P * G
C = 8            # classes
BPC = 4          # images per core
NCORES = 8
MBLK = 16        # GTs per block
MAXBLK = 4
NQ = 5
NOUT = 6

KPACK = float(2 ** 22)
C29 = float(2 ** 29)
TPOS = KPACK * 0.2 + 31.5
TNEG = KPACK / 11.0 + 31.5

CHUNKS = ((0, 72), (72, 64), (136, 64))   # (group base, group count)
import os
BATCH_T = os.environ.get("DETLOSS_BATCH_T", "1") == "1"

# anchor plane order (pairs/runs must stay adjacent for packed ops):
# [WH, HH, CXM, CYM, W, H, PX, PY, LW, LH, I2W, I2H, ONE, ONE]
A_WH, A_HH, A_CXM, A_CYM, A_W, A_H, A_PX, A_PY, A_LW, A_LH, \
    A_I2W, A_I2H, A_ONE1, A_ONE2 = range(14)
NANC = 14
G_X1, G_Y1, G_X2, G_Y2, G_A2K = range(5)
NGT = 5


def _rep_last(ap, n):
    return bass.AP(ap.tensor, ap.offset, list(ap.ap) + [[0, n]])


def _rep_mid(ap, n):
    a = list(ap.ap)
    return bass.AP(ap.tensor, ap.offset, [a[0], [0, n]] + a[1:])


def _stride5(ap2d, q):
    """cand [128, 1000] -> [128, 200] plane of quantity q."""
    a = ap2d
    return bass.AP(a.tensor, a.offset + q, [a.ap[0], [NQ, G]])


def build_program(slot_blocks):
    nc = bacc.Bacc(None, target_bir_lowering=False)
    cls_d = nc.dram_tensor("cls", [BPC, P, C * G], F32, kind="ExternalInput")
    reg_d = nc.dram_tensor("reg", [BPC, P, 4 * G], F32, kind="ExternalInput")
    anc_d = nc.dram_tensor("anc", [P, NANC * G], F32, kind="ExternalInput")
    gtp_d = nc.dram_tensor("gtp", [BPC, MAXBLK, P, NGT * MBLK], F32,
                           kind="ExternalInput")
    gtq_d = nc.dram_tensor("gtq", [BPC, MAXBLK, P, 8 * NQ], F16,
                           kind="ExternalInput")
    iota_d = nc.dram_tensor("iota", [P, MAXBLK * MBLK], F32, kind="ExternalInput")
    iotc_d = nc.dram_tensor("iotc", [P, C * G], F32, kind="ExternalInput")
    res_d = nc.dram_tensor("res", [BPC, P, NOUT], F32, kind="ExternalOutput")

    rk = RECIP_APPROX_FAST_CONSTS

    with tile.TileContext(nc) as tc:
        with (
            tc.tile_pool(name="const", bufs=1) as cpool,
            tc.tile_pool(name="img", bufs=2) as ipool,
            tc.tile_pool(name="blk", bufs=2) as bpool,
            tc.tile_pool(name="chk", bufs=2) as kpool,
            tc.tile_pool(name="psum", bufs=2, space="PSUM") as ppool,
            tc.tile_pool(name="out", bufs=2) as opool,
        ):
            anc = cpool.tile([P, NANC * G], F32)
            nc.sync.dma_start(anc[:], anc_d[:])
            iota = cpool.tile([P, MAXBLK * MBLK], F32)
            nc.sync.dma_start(iota[:], iota_d[:])
            iotc = cpool.tile([P, C * G], F32)
            nc.sync.dma_start(iotc[:], iotc_d[:])

            def ancp(k, n=1):
                return anc[:, k * G:(k + n) * G]

            for b in range(BPC):
                nblk = slot_blocks[b]
                ct = ipool.tile([P, C * G], F32, tag="ct", name="ct")
                nc.sync.dma_start(ct[:], cls_d[b])
                rt = ipool.tile([P, 4 * G], F32, tag="rt", name="rt")
                nc.sync.dma_start(rt[:], reg_d[b])

                def it(tag, n=1):
                    return ipool.tile([P, n * G], F32, tag=tag, name=tag)

                def clsp(k):
                    return ct[:, k * G:(k + 1) * G]

                # ---- decode (packed planes) ----
                cxy = it("cxy", 2)      # [cx | cy]
                wh2 = it("wh2", 2)      # [w | h]
                p1 = it("p1", 2)        # [x1 | y1]
                p2 = it("p2", 2)        # [x2 | y2]
                a1K = it("a1K")
                rp4 = it("rp4", 4)      # reg + [PX PY LW LH]
                nc.vector.tensor_tensor(cxy[:], rt[:, 0:2 * G],
                                        ancp(A_WH, 2), ALU.mult)
                nc.gpsimd.tensor_tensor(cxy[:], cxy[:], ancp(A_CXM, 2), ALU.add)
                nc.scalar.activation(wh2[:], rt[:, 2 * G:4 * G], ACT.Exp)
                nc.vector.tensor_tensor(wh2[:], wh2[:], ancp(A_W, 2), ALU.mult)
                nc.vector.scalar_tensor_tensor(p1[:], wh2[:], -0.5, cxy[:],
                                               ALU.mult, ALU.add)
                nc.vector.scalar_tensor_tensor(p2[:], wh2[:], 0.5, cxy[:],
                                               ALU.mult, ALU.add)
                nc.vector.scalar_tensor_tensor(a1K[:], wh2[:, 0:G], 2.0 ** -22,
                                               wh2[:, G:2 * G], ALU.mult,
                                               ALU.mult)
                nc.gpsimd.tensor_tensor(rp4[:], rt[:], ancp(A_PX, 4), ALU.add)
                x1 = p1[:, 0:G]; y1 = p1[:, G:2 * G]
                x2 = p2[:, 0:G]; y2 = p2[:, G:2 * G]

                zmax_run = ipool.tile([P, G], F32, tag="zmax_run",
                                      name="zmax_run")
                cand_run = ipool.tile([P, NQ * G], F32, tag="cand_run",
                                      name="cand_run")
                if nblk == 0:
                    nc.gpsimd.memset(zmax_run[:], 0.0)
                    nc.gpsimd.memset(cand_run[:], 0.0)

                # ---- GT blocks ----
                for blk in range(nblk):
                    gtt = bpool.tile([P, NGT * MBLK], F32, tag="gtt", name="gtt")
                    nc.sync.dma_start(gtt[:], gtp_d[b, blk])
                    gtqt = bpool.tile([P, 8 * NQ], F16, tag="gtqt", name="gtqt")
                    nc.sync.dma_start(gtqt[:], gtq_d[b, blk])
                    if blk == 0:
                        zmax_blk, cand_blk = zmax_run, cand_run
                    else:
                        zmax_blk = bpool.tile([P, G], F32, tag="zmax_blk",
                                              name="zmax_blk")
                        cand_blk = bpool.tile([P, NQ * G], F32, tag="cand_blk",
                                              name="cand_blk")

                    def gtp(k):
                        return gtt[:, k * MBLK:(k + 1) * MBLK]

                    for (g0, gc) in CHUNKS:
                        cols = gc * MBLK
                        units = cols // 128
                        sl = slice(g0, g0 + gc)

                        def kt(tag):
                            return kpool.tile([P, CHUNKS[0][1] * MBLK], F32,
                                              tag=tag, name=tag)

                        tA = kt("tA"); tB = kt("tB"); tC = kt("tC")
                        tD = kt("tD")
                        mask = kpool.tile([P, CHUNKS[0][1] * MBLK], F16,
                                          tag="mask", name="mask")
                        maskT = kpool.tile([P, CHUNKS[0][1] * MBLK], F16,
                                           tag="maskT", name="maskT")
                        vA = tA[:, 0:cols].rearrange("p (g m) -> p g m", m=MBLK)
                        vB = tB[:, 0:cols].rearrange("p (g m) -> p g m", m=MBLK)
                        vC = tC[:, 0:cols].rearrange("p (g m) -> p g m", m=MBLK)
                        vD = tD[:, 0:cols].rearrange("p (g m) -> p g m", m=MBLK)

                        nc.vector.tensor_tensor(vA, _rep_last(x1[:, sl], MBLK),
                                                _rep_mid(gtp(G_X1), gc), ALU.max)
                        nc.vector.tensor_tensor(vB, _rep_last(x2[:, sl], MBLK),
                                                _rep_mid(gtp(G_X2), gc), ALU.min)
                        nc.gpsimd.tensor_tensor(vC, vB, vA, ALU.subtract)  # iwr
                        nc.vector.tensor_tensor(vA, _rep_last(y1[:, sl], MBLK),
                                                _rep_mid(gtp(G_Y1), gc), ALU.max)
                        nc.vector.tensor_tensor(vB, _rep_last(y2[:, sl], MBLK),
                                                _rep_mid(gtp(G_Y2), gc), ALU.min)
                        nc.gpsimd.tensor_tensor(vD, vB, vA, ALU.subtract)  # ihr
                        nc.scalar.activation(tB[:, 0:cols], tD[:, 0:cols],
                                             ACT.Relu)                     # ihp
                        nc.scalar.activation(tA[:, 0:cols], tC[:, 0:cols],
                                             ACT.Relu)                     # iwp
                        nc.gpsimd.tensor_tensor(vC, vA, vB, ALU.mult)      # inter
                        nc.gpsimd.tensor_tensor(
                            vA, _rep_last(a1K[:, sl], MBLK),
                            _rep_mid(gtp(G_A2K), gc), ALU.add)             # s'
                        nc.vector._custom_dve(RECIPROCAL_APPROX_FAST,
                                              out=tD[:, 0:cols],
                                              in0=tA[:, 0:cols],
                                              s0=rk["s0"], s1=rk["s1"],
                                              imm2=rk["imm2"])             # rcp
                        nc.vector.scalar_tensor_tensor(vB, vC, 1.0, vD,
                                                       ALU.mult, ALU.mult)  # zK
                        nc.scalar.activation(tA[:, 0:cols], tB[:, 0:cols],
                                             ACT.Copy, bias=C29)           # q1
                        nc.scalar.activation(tD[:, 0:cols], tA[:, 0:cols],
                                             ACT.Copy, bias=-C29)          # q2
                        nc.gpsimd.tensor_tensor(
                            vA, vD,
                            _rep_mid(iota[:, blk * MBLK:(blk + 1) * MBLK], gc),
                            ALU.add)                                       # z
                        nc.vector.reduce_max(zmax_blk[:, sl], vA, axis=AX.X)
                        nc.vector.tensor_tensor(
                            mask[:, 0:cols].rearrange("p (g m) -> p g m",
                                                      m=MBLK),
                            vA, _rep_last(zmax_blk[:, sl], MBLK), ALU.is_equal)

                        # batched transpose + fp16 matmuls
                        if BATCH_T:
                            nc.sync.dma_start_transpose(
                                maskT[:, 0:cols].rearrange("p (u c) -> p u c",
                                                           c=128),
                                mask[:, 0:cols])
                        ps = ppool.tile([P, NQ * CHUNKS[0][1]], F32, tag="ps",
                                        name="ps")
                        for u in range(units):
                            usl = slice(u * 128, (u + 1) * 128)
                            if not BATCH_T:
                                nc.sync.dma_start_transpose(maskT[:, usl],
                                                            mask[:, usl])
                            nc.tensor.matmul(ps[:, u * 40:(u + 1) * 40],
                                             maskT[:, usl], gtqt[:])
                        nc.scalar.activation(
                            cand_blk[:, NQ * g0:NQ * (g0 + gc)],
                            ps[:, 0:NQ * gc], ACT.Copy)

                    if blk > 0:
                        bsel5 = bpool.tile([P, NQ * G], U8, tag="bsel5",
                                           name="bsel5")
                        nc.vector.tensor_tensor(
                            bsel5[:].rearrange("p (g q) -> p g q", q=NQ),
                            _rep_last(zmax_blk[:], NQ),
                            _rep_last(zmax_run[:], NQ), ALU.is_gt)
                        nc.vector.copy_predicated(cand_run[:], bsel5[:],
                                                  cand_blk[:])
                        nc.vector.tensor_tensor(zmax_run[:], zmax_run[:],
                                                zmax_blk[:], ALU.max)

                # ---- thresholds ----
                posf = it("posf"); negf = it("negf")
                nc.vector.tensor_scalar(posf[:], zmax_run[:], TPOS, None,
                                        ALU.is_ge)
                nc.vector.tensor_scalar(negf[:], zmax_run[:], TNEG, None,
                                        ALU.is_lt)

                # ---- classification ----
                ext = ipool.tile([P, C * G], F32, tag="ext", name="ext")
                nc.scalar.activation(ext[:], ct[:], ACT.Exp)
                sumex = it("sumex"); lse = it("lse"); bgt = it("bgt")
                xt = it("xt")
                nc.vector.reduce_sum(
                    sumex[:],
                    bass.AP(ext[:].tensor, ext[:].offset,
                            [ext[:].ap[0], [1, G], [G, C]]),
                    axis=AX.X)
                nc.scalar.activation(lse[:], sumex[:], ACT.Ln)
                nc.gpsimd.tensor_tensor(bgt[:], lse[:], clsp(0), ALU.subtract)
                # xt = sum_c [lab == c] * x_c  via wide is_eq + mult + reduce
                lab = _stride5(cand_run[:], 4)
                ind8 = ipool.tile([P, C * G], F32, tag="ind8", name="ind8")
                nc.vector.tensor_tensor(
                    ind8[:],
                    bass.AP(lab.tensor, lab.offset,
                            [lab.ap[0], [0, C], [NQ, G]]),
                    iotc[:], ALU.is_equal)
                nc.gpsimd.tensor_tensor(ind8[:], ind8[:], ct[:], ALU.mult)
                nc.vector.reduce_sum(
                    xt[:],
                    bass.AP(ind8[:].tensor, ind8[:].offset,
                            [ind8[:].ap[0], [1, G], [G, C]]),
                    axis=AX.X)
                tgtt = it("tgtt")
                nc.gpsimd.tensor_tensor(tgtt[:], lse[:], xt[:], ALU.subtract)

                # ---- regression smooth-L1 (packed 4 planes) ----
                m4 = it("m4", 4); d4 = it("d4", 4); ad4 = it("ad4", 4)
                zc4 = it("zc4", 4)
                cand4 = bass.AP(cand_run[:].tensor, cand_run[:].offset,
                                [cand_run[:].ap[0], [1, 4], [NQ, G]])
                nc.vector.tensor_tensor(
                    m4[:].rearrange("p (q g) -> p q g", g=G), cand4,
                    ancp(A_I2W, 4).rearrange("p (q g) -> p q g", g=G), ALU.mult)
                nc.gpsimd.tensor_tensor(d4[:], rp4[:], m4[:], ALU.subtract)
                nc.scalar.activation(ad4[:], d4[:], ACT.Abs)
                nc.vector.tensor_scalar(zc4[:], ad4[:], 1.0, None, ALU.min)
                nc.vector.scalar_tensor_tensor(d4[:], zc4[:], -0.5, ad4[:],
                                               ALU.mult, ALU.add)   # |d|-z/2
                # h4 written in (g, q)-major layout so the reduce is innermost
                h4 = it("h4", 4)

                def _gq(t):
                    a = t[:]
                    return bass.AP(a.tensor, a.offset,
                                   [a.ap[0], [1, G], [G, 4]])
                nc.gpsimd.tensor_tensor(
                    h4[:].rearrange("p (g q) -> p g q", q=4),
                    _gq(zc4), _gq(d4), ALU.mult)
                nsl = it("nsl")
                nc.vector.reduce_sum(
                    nsl[:], h4[:].rearrange("p (g q) -> p g q", q=4), axis=AX.X)

                # ---- output partials ----
                ot = opool.tile([P, NOUT], F32, tag="ot", name="ot")
                scr = it("scr")
                nc.scalar.activation(scr[:], posf[:], ACT.Copy,
                                     accum_out=ot[:, 0:1])
                nc.scalar.activation(scr[:], negf[:], ACT.Copy,
                                     accum_out=ot[:, 1:2])
                nc.scalar.activation(scr[:], bgt[:], ACT.Copy,
                                     accum_out=ot[:, 2:3])
                scr2 = it("scr2")
                nc.vector.tensor_tensor(scr[:], tgtt[:], posf[:], ALU.mult)
                nc.scalar.activation(scr2[:], scr[:], ACT.Copy,
                                     accum_out=ot[:, 3:4])
                nc.vector.tensor_tensor(scr[:], bgt[:], negf[:], ALU.mult)
                nc.scalar.activation(scr2[:], scr[:], ACT.Copy,
                                     accum_out=ot[:, 4:5])
                nc.gpsimd.tensor_tensor(scr[:], nsl[:], posf[:], ALU.mult)
                nc.scalar.activation(scr2[:], scr[:], ACT.Copy, scale=-1.0,
                                     accum_out=ot[:, 5:6])

                nc.sync.dma_start(res_d[b], ot[:])
    nc.compile()
    return nc


_NC_CACHE = {}


def _get_nc(slot_blocks):
    key = tuple(slot_blocks)
    if key not in _NC_CACHE:
        _NC_CACHE[key] = build_program(key)
    return _NC_CACHE[key]


def plan_assignment(num_boxes):
    nb = np.asarray(num_boxes)
    blocks = np.ceil(nb / MBLK).astype(int)
    order = np.argsort(-blocks, kind="stable")
    assign = np.zeros((NCORES, BPC), int)
    for rank, img in enumerate(order):
        assign[rank % NCORES, rank // NCORES] = img
    slot_blocks = tuple(
        int(max(blocks[assign[c, s]] for c in range(NCORES)))
        for s in range(BPC))
    return assign, slot_blocks


def prep_inputs(cls_output, reg_output, anchors, gt_boxes, gt_labels, num_boxes):
    B = cls_output.shape[0]
    cls_output = np.asarray(cls_output, np.float32)
    reg_output = np.asarray(reg_output, np.float32)
    anchors = np.asarray(anchors, np.float32)
    gt_boxes = np.asarray(gt_boxes, np.float32)
    gt_labels = np.asarray(gt_labels)
    num_boxes = np.asarray(num_boxes)

    aw = anchors[:, 2] - anchors[:, 0]
    ah = anchors[:, 3] - anchors[:, 1]
    acx = anchors[:, 0] + 0.5 * aw
    acy = anchors[:, 1] + 0.5 * ah
    i2w = 2.0 / aw
    i2h = 2.0 / ah
    ones = np.ones_like(aw)
    planes = np.stack([
        aw / 2.0, ah / 2.0, acx - aw / 4.0, acy - ah / 4.0, aw, ah,
        acx * i2w - 0.5, acy * i2h - 0.5, np.log(aw), np.log(ah),
        i2w, i2h, ones, ones,
    ], axis=0).astype(np.float32)
    anc_host = planes.reshape(NANC, P, G).transpose(1, 0, 2).reshape(P, NANC * G)

    assign, slot_blocks = plan_assignment(num_boxes)

    cls_host = cls_output.reshape(B, C, P, G).transpose(0, 2, 1, 3) \
        .reshape(B, P, C * G)
    reg_host = reg_output.reshape(B, 4, P, G).transpose(0, 2, 1, 3) \
        .reshape(B, P, 4 * G)

    iota_host = np.broadcast_to(
        np.arange(MAXBLK * MBLK, dtype=np.float32)[None, :],
        (P, MAXBLK * MBLK)).copy()
    iotc_host = np.broadcast_to(
        np.repeat(np.arange(C, dtype=np.float32), G)[None, :],
        (P, C * G)).copy()

    gw = gt_boxes[..., 2] - gt_boxes[..., 0]
    gh = gt_boxes[..., 3] - gt_boxes[..., 1]
    gcx = gt_boxes[..., 0] + 0.5 * gw
    gcy = gt_boxes[..., 1] + 0.5 * gh
    lgw = np.log(np.maximum(gw, 1e-6))
    lgh = np.log(np.maximum(gh, 1e-6))
    area = (gw * gh).astype(np.float32)

    def image_blocks(img):
        nbx = int(num_boxes[img])
        gtp = np.zeros((MAXBLK, NGT, MBLK), np.float32)
        q5 = np.zeros((MAXBLK, MBLK, NQ), np.float16)
        for m in range(min(nbx, 50)):
            blk, mm = divmod(m, MBLK)
            gtp[blk, G_X1, mm] = gt_boxes[img, m, 0]
            gtp[blk, G_Y1, mm] = gt_boxes[img, m, 1]
            gtp[blk, G_X2, mm] = gt_boxes[img, m, 2]
            gtp[blk, G_Y2, mm] = gt_boxes[img, m, 3]
            gtp[blk, G_A2K, mm] = area[img, m] * np.float32(2.0 ** -22)
            q5[blk, mm] = np.float16([gcx[img, m], gcy[img, m], lgw[img, m],
                                      lgh[img, m], float(gt_labels[img, m])])
        gtq = np.zeros((MAXBLK, P, 8 * NQ), np.float16)
        for gp in range(8):
            gtq[:, gp * MBLK:(gp + 1) * MBLK, gp * NQ:(gp + 1) * NQ] = q5
        gtp_flat = np.broadcast_to(
            gtp.reshape(MAXBLK, 1, NGT * MBLK), (MAXBLK, P, NGT * MBLK))
        return gtp_flat.astype(np.float32), gtq

    in_maps = []
    for core in range(NCORES):
        imgs = [int(assign[core, s]) for s in range(BPC)]
        gtp_c = np.zeros((BPC, MAXBLK, P, NGT * MBLK), np.float32)
        gtq_c = np.zeros((BPC, MAXBLK, P, 8 * NQ), np.float16)
        for s, img in enumerate(imgs):
            gtp_c[s], gtq_c[s] = image_blocks(img)
        in_maps.append({
            "cls": np.ascontiguousarray(cls_host[imgs]),
            "reg": np.ascontiguousarray(reg_host[imgs]),
            "anc": anc_host,
            "gtp": gtp_c,
            "gtq": gtq_c,
            "iota": iota_host,
            "iotc": iotc_host,
        })
    return in_maps, num_boxes, assign, slot_blocks


def finish(res_all, num_boxes):
    s = res_all.sum(axis=1).astype(np.float32)
    npos, nneg, ce_bg_sum, ce_tgt_pos, ce_bg_neg, neg_sl = (s[:, i]
                                                            for i in range(6))
    sl_pos = -neg_sl
    has = num_boxes > 0
    cls_pos = np.where(npos > 0, ce_tgt_pos / np.maximum(npos, 1.0), 0.0)
    cls_neg = np.where(nneg > 0, ce_bg_neg / np.maximum(nneg, 1.0), 0.0)
    cls_losses = np.where(has, cls_pos + cls_neg, ce_bg_sum / np.float32(N))
    reg_losses = np.where(npos > 0, sl_pos / np.maximum(npos * 4.0, 1.0), 0.0)
    total_pos = npos.sum(dtype=np.float32)
    cls_final = np.float32(cls_losses.astype(np.float32).mean())
    reg_final = np.float32(reg_losses.astype(np.float32).sum()
                           / max(total_pos, 1.0))
    return (np.float32(cls_final + reg_final), cls_final, reg_final,
            np.float32(total_pos))


def kernel(cls_output, reg_output, anchors, gt_boxes, gt_labels, num_boxes):
    in_maps, num_boxes, assign, slot_blocks = prep_inputs(
        cls_output, reg_output, anchors, gt_boxes, gt_labels, num_boxes)
    nc = _get_nc(slot_blocks)
    out = run_bass_kernel_spmd(nc, in_maps, list(range(NCORES)))
    res_perm = np.concatenate([np.asarray(r["res"]) for r in out.results],
                              axis=0)
    res_all = np.zeros_like(res_perm)
    for pos, img in enumerate(assign.reshape(-1)):
        res_all[img] = res_perm[pos]
    return finish(res_all, np.asarray(num_boxes))
